# revision 1
# baseline (speedup 1.0000x reference)
"""NativeFP4Linear TRN2 kernel: out = x @ (dequant(weight_fp4)).T + bias.

dequant(W)[o, i] = W[o, i] / block_scales[o*256 + i//16] / tensor_scale

Strategy (8 NeuronCores, tensor-parallel over out_features, 512 rows/core):
  - Host: transpose each core's weight slice to [in=4096, out=512] (matmul
    contracts along the partition dim, so the weight must be partition=i).
  - Device per core:
      rec   = 1/block_scales  (DVE reciprocal_approx_fast, [128, 1024] layout)
      rec   -> hi + lo fp32r pieces (exact sum)
      ES    = one-hot fp32r matmuls broadcast rec rows into [128 i, 512 o]
              expanded-scale tiles (hi+lo accumulated -> bit-exact fp32 scales)
      wdeq  = wT * ES           (DVE tensor-tensor, fp32 -> fp32r)
      out  += xT_chunk.T @ wdeq (fp32r matmuls, K accumulated in PSUM fp32)
      out   = out * (1/tensor_scale) + bias
  - Host: concatenate the 8 [32, 512] results -> [32, 4096].
"""
import numpy as np
from contextlib import ExitStack

import concourse.bass as bass
import concourse.mybir as mybir
import concourse.tile as tile
from concourse import bacc
from concourse.bass_utils import run_bass_kernel_spmd

F32 = mybir.dt.float32
F32R = mybir.dt.float32r
BF16 = mybir.dt.bfloat16
U8 = mybir.dt.uint8

N_CORES = 8
B = 32             # batch
I = 4096           # in_features
O = 4096           # out_features
OC = O // N_CORES  # out features per core = 512
BS = 16            # fp4 block size
NBLK = I // BS     # block-columns per output row = 256
NSUB = I // 128    # 128-row contraction sub-chunks = 32
SUB_PER_IT = 3     # sub-chunks fused per pipeline iteration
PREFETCH = 8

_CACHE = {}


def _build(inv_ts: float):
    nc = bacc.Bacc("TRN2", target_bir_lowering=False, debug=False,
                   enable_asserts=True, num_devices=N_CORES)

    wt = nc.dram_tensor("wt", [I, OC], F32, kind="ExternalInput").ap()
    sc = nc.dram_tensor("sc", [128, 2048], F32, kind="ExternalInput").ap()
    e16in = nc.dram_tensor("e16", [128, 8 * 128], BF16,
                           kind="ExternalInput").ap()
    # combo holds xt (fp32, 4096 B/partition)
    combo = nc.dram_tensor("combo", [128, 4096], U8, kind="ExternalInput").ap()
    biasb = nc.dram_tensor("biasb", [B, OC], F32, kind="ExternalInput").ap()
    out = nc.dram_tensor("out", [B, OC], F32, kind="ExternalOutput").ap()

    with tile.TileContext(nc) as tc, ExitStack() as ctx:
        cpool = ctx.enter_context(tc.tile_pool(name="const", bufs=1))
        wpool = ctx.enter_context(tc.tile_pool(name="w", bufs=10))
        dqpool = ctx.enter_context(tc.tile_pool(name="dq", bufs=4))
        espool = ctx.enter_context(tc.tile_pool(name="es", bufs=2, space="PSUM"))
        mpool = ctx.enter_context(tc.tile_pool(name="acc", bufs=1, space="PSUM"))

        # First iteration small so the first dequant starts early; last
        # iteration tiny so little work trails the final weight DMA.
        sizes = [1, 2] + [SUB_PER_IT] * 9 + [1, 1]
        assert sum(sizes) == NSUB
        starts = [sum(sizes[:i]) for i in range(len(sizes))]
        n_it = len(starts)

        # ---- setup DMAs. sc rides the Sync HWDGE ring first; the other
        # small setup tensors go through SWDGE (gpsimd) so their issue cost
        # doesn't serialize against the weight stream, and the Scalar
        # engine stays free for compute (its sequencer would otherwise
        # stall casts behind DMA descriptor generation). ----
        t_sc = cpool.tile([128, 2048], F32)
        nc.sync.dma_start(t_sc[:, :1024], sc[:, :1024])
        sc2_inst = nc.sync.dma_start(t_sc[:, 1024:], sc[:, 1024:])
        t_e16bf = cpool.tile([128, 8 * 128], BF16)
        nc.gpsimd.dma_start(t_e16bf[:], e16in[:])
        t_combo = cpool.tile([128, 4096], U8)
        combo_inst = nc.gpsimd.dma_start(t_combo[:], combo[:])
        t_biasb = cpool.tile([B, OC], F32)
        nc.gpsimd.dma_start(t_biasb[:], biasb[:])

        def dma_w(t):
            # Weight DMAs ride the Sync HWDGE ring. All but the first are
            # held until the setup data has landed: the SDMA engines
            # round-robin queues at packet granularity, so concurrent bulk
            # weight traffic would starve the small setup transfers that
            # gate the whole compute pipeline.
            g, nsc = starts[t], sizes[t]
            t_w = wpool.tile([128, SUB_PER_IT * OC], F32, tag="w")
            src = wt[g * 128:g * 128 + nsc * 128, :].rearrange(
                "(q p) n -> p q n", p=128)
            inst = nc.sync.dma_start(t_w[:, :nsc * OC].rearrange(
                "p (q n) -> p q n", q=nsc), src)
            if t > 0:
                tile.add_dep_helper(inst.ins, combo_inst.ins,
                                    reason="hold bulk weights behind setup")
            return t_w

        tile.add_dep_helper(sc2_inst.ins, combo_inst.ins,
                            reason="hold sc tail behind setup head")

        w_tiles = [dma_w(t) for t in range(min(PREFETCH, n_it))]

        # ---- e16 cast (ScalarE; parallel with the DVE reciprocal work) ----
        t_e16 = cpool.tile([128, 8 * 128], F32R)
        nc.scalar.copy(t_e16[:], t_e16bf[:])

        # ---- reciprocal chain. sc holds the scales twice (partitions
        # 0-63 == 64-127); rhl packs fp32r hi rows in partitions 0-63 and
        # the lo residual in 64-127 so ONE one-hot matmul per sub-chunk
        # expands exact-fp32 scales. Split in column quarters so the first
        # expansion matmuls only wait for the first quarter; the fp32r hi
        # rounding writes the whole tile and the lo subtract then
        # overwrites partitions 64-127 in place. ----
        t_rhl = cpool.tile([128, 2048], F32R)
        t_rec = cpool.tile([128, 2048], F32)
        prev = None
        for h in range(4):
            cols = slice(512 * h, 512 * (h + 1))
            i0 = nc.vector.reciprocal_approx_fast(t_rec[:, cols],
                                                  t_sc[:, cols])
            if prev is not None:
                # ordering-only dep: keep later quarters from being
                # scheduled ahead of earlier ones on the DVE
                tile.add_dep_helper(i0.ins, prev.ins, sync=False,
                                    reason="dve setup order")
            if h == 0:
                nc.vector.tensor_copy(t_rhl[:, cols], t_rec[:, cols])
            else:
                nc.scalar.copy(t_rhl[:, cols], t_rec[:, cols])
            prev = nc.vector.tensor_sub(t_rhl[64:, cols], t_rec[64:, cols],
                                        t_rhl[64:, cols].bitcast(F32))

        t_xtr = cpool.tile([128, NSUB * B], F32R)
        nc.scalar.copy(t_xtr[:], t_combo[:].bitcast(F32))

        t_acc = mpool.tile([B, OC], F32)

        def emit_es(t):
            g, nsc = starts[t], sizes[t]
            t_es = espool.tile([128, SUB_PER_IT * OC], F32, tag="es")
            for j in range(nsc):
                gg = g + j
                m, u = gg % 8, gg // 8
                lhs = t_e16[:, 128 * m:128 * (m + 1)]
                dst = t_es[:, OC * j:OC * (j + 1)]
                nc.tensor.matmul(dst, lhs, t_rhl[:, OC * u:OC * (u + 1)],
                                 start=True, stop=True)
            return t_es

        # ---- software-pipelined main loop ----
        # PE order: ES(t+1) is emitted before main(t) so the tensor engine
        # fills the DVE-dequant latency with the next chunk's expansion.
        es_tiles = {0: emit_es(0)}
        for t in range(n_it):
            g, nsc = starts[t], sizes[t]
            if t + PREFETCH < n_it:
                w_tiles.append(dma_w(t + PREFETCH))
            if t + 1 < n_it:
                es_tiles[t + 1] = emit_es(t + 1)

            t_es = es_tiles.pop(t)
            t_w = w_tiles[t]
            t_dq = dqpool.tile([128, SUB_PER_IT * OC], F32R, tag="dq")
            nc.vector.tensor_mul(t_dq[:, :nsc * OC], t_w[:, :nsc * OC],
                                 t_es[:, :nsc * OC])

            for j in range(nsc):
                gg = g + j
                nc.tensor.matmul(t_acc[:], t_xtr[:, B * gg:B * (gg + 1)],
                                 t_dq[:, OC * j:OC * (j + 1)],
                                 start=(gg == 0), stop=(gg == NSUB - 1))

        # ---- epilogue: out = acc * (1/ts) + bias ----
        t_out = cpool.tile([B, OC], F32)
        nc.vector.scalar_tensor_tensor(
            t_out[:], t_acc[:], float(inv_ts), t_biasb[:],
            op0=mybir.AluOpType.mult, op1=mybir.AluOpType.add)
        nc.sync.dma_start(out[:], t_out[:])

    nc.compile()
    return nc


def _host_prep(x, weight_fp4, block_scales, bias):
    """Build the per-core input maps."""
    import ml_dtypes
    x = np.asarray(x, dtype=np.float32)
    weight_fp4 = np.asarray(weight_fp4, dtype=np.float32)
    block_scales = np.asarray(block_scales, dtype=np.float32)
    bias = np.asarray(bias, dtype=np.float32)

    # x.T tiled: xt[p, 32 g + b] = x[b, 128 g + p]
    xt = np.ascontiguousarray(
        x.T.reshape(NSUB, 128, B).transpose(1, 0, 2).reshape(128, NSUB * B))

    # one-hot selectors picking the hi row (partition 8m+p//16) and the
    # lo row (64 + 8m + p//16) of the packed reciprocal tile
    e16 = np.zeros((128, 8 * 128), dtype=ml_dtypes.bfloat16)
    for m in range(8):
        p = np.arange(128)
        e16[8 * m + p // 16, 128 * m + p] = 1.0
        e16[64 + 8 * m + p // 16, 128 * m + p] = 1.0

    combo = np.ascontiguousarray(xt.view(np.uint8).reshape(128, 4096))

    bs2 = block_scales.reshape(O, NBLK)

    in_maps = []
    for c in range(N_CORES):
        o0 = c * OC
        wt_c = np.ascontiguousarray(weight_fp4[o0:o0 + OC, :].T)
        s_core = bs2[o0:o0 + OC, :].T  # [256 blk, 512 o]
        # sc_c[p, 512 j + n] = s_core[64 j + (p mod 64), n]; halves duplicated
        h64 = s_core.reshape(4, 64, OC).transpose(1, 0, 2).reshape(64, 4 * OC)
        sc_c = np.ascontiguousarray(np.concatenate([h64, h64], axis=0))
        biasb_c = np.ascontiguousarray(
            np.broadcast_to(bias[o0:o0 + OC][None, :], (B, OC)))
        in_maps.append({
            "wt": wt_c, "sc": sc_c, "e16": e16, "combo": combo,
            "biasb": biasb_c,
        })
    return in_maps


def _get_program(inv_ts: float):
    key = ("nc", float(inv_ts))
    if key not in _CACHE:
        _CACHE[key] = _build(inv_ts)
    return _CACHE[key]


def kernel(x, weight_fp4, tensor_scale, block_scales, bias, **run_kwargs):
    inv_ts = 1.0 / float(np.asarray(tensor_scale).reshape(-1)[0])
    nc = _get_program(inv_ts)
    in_maps = _host_prep(x, weight_fp4, block_scales, bias)
    res = run_bass_kernel_spmd(nc, in_maps, core_ids=list(range(N_CORES)),
                               **run_kwargs)
    out = np.empty((B, O), dtype=np.float32)
    for c in range(N_CORES):
        out[:, c * OC:(c + 1) * OC] = res.results[c]["out"]
    if run_kwargs.get("trace"):
        kernel.last_exec_time_ns = res.exec_time_ns
    return out

